# revision 87
# baseline (speedup 1.0000x reference)
"""CQAttention (QANet context-query attention) Bass kernel for 8 Trainium2 cores.

Math (per batch, masks all-ones, eval mode):
  Ct = C.T [Lc,D], Qt = Q.T [Lq,D]
  S  = Ct@w4C + (Qt@w4Q).T + (Ct*w4mlu)@Qt.T + bias          [Lc,Lq]
  S1 = softmax_q(S), S2 = softmax_c(S)
  A  = S1@Qt ; Bt = S1@(S2.T@Ct)
  out = concat([Ct, A, Ct*A, Ct*Bt], -1).T                    [4D, Lc]

Key reductions used here:
  - (S1@S2.T)@Ct re-associated as S1@(S2.T@Ct)  (6x fewer flops)
  - softmax terms constant along the reduced axis cancel, so:
      S1 = E1/r,  E1^T[q,c] = exp(sum_d Q[d,q]*Caug[d,c]),  Caug = C*w4mlu + w4Q
      S2 = E2/s,  E2[c,q]   = exp(sum_d C[d,c]*Qaug[d,q]),  Qaug = Q*w4mlu + w4C
    (bias and the remaining rank-1 terms cancel exactly in every output)
  - row-sums r replicated across partitions for free via ones-matmul
  - outputs stay in [d, c] layout end-to-end:
      out1 = MA*(1/r), out2 = MA*(C/r), out3 = MB*(C/r)
      MA = Qt.T @ E1^T, MB = T.T @ E1^T, T = (Ct.T @ E2).T * (1/s)

Scheduling (v9):
  - C/Q DMA straight into float32r tiles (same bits as f32): no rounding
    copies, and f32r transposes run 1.5 cyc/row.
  - out1/out2 (the MA planes, 2/3 of the store bytes) depend only on
    E1 + r, not on the S2/T chain, so they are computed and stored before
    the T chain finishes; out3 streams right behind MB in 512-col chunks
    on the small psum pool.
  - two-batch software pipeline: batch b+1's head stages (Qt + first E2
    groups, then E1 + last E2 group, then r) are emitted INSIDE batch b's
    T-chain latency windows, so the PE always has an independent
    instruction stream while Act drains the exp queue.
  - engine placement: exps + Ct/Qt/s_sb copies on Act; rbi reciprocals,
    MTt copy, T scale and all psum-reading output muls on DVE (gpsimd
    cannot touch PSUM on hardware); Crbi on gpsimd; all DMAs issued from
    SP with next-batch inputs emitted ahead of current-batch stores.
"""

import numpy as np

import concourse.bass as bass
import concourse.bacc as bacc
import concourse.tile as tile
from concourse import mybir
from contextlib import ExitStack, nullcontext

B, D, LC, LQ = 32, 128, 2048, 256
NCORES = 8
BPC = B // NCORES  # batches per core

F32 = mybir.dt.float32
F32R = mybir.dt.float32r
AF = mybir.ActivationFunctionType
ALU = mybir.AluOpType

# pool-depth tuning knobs
IO_BUFS = 2
BIG_BUFS = 3
SMALL_BUFS = 2
WORK_BUFS = 2
OUT_CHUNKS = 2  # store chunks per batch per plane-pair


def build_nc(reps=1, hw_loop=False):
    nc = bacc.Bacc("TRN2", target_bir_lowering=False)
    C_in = nc.declare_dram_parameter("C", [BPC, D, LC], F32R, isOutput=False)
    Q_in = nc.declare_dram_parameter("Q", [BPC, D, LQ], F32R, isOutput=False)
    w4C_in = nc.declare_dram_parameter("w4C", [D, 1], F32, isOutput=False)
    w4Q_in = nc.declare_dram_parameter("w4Q", [D, 1], F32, isOutput=False)
    w4mlu_in = nc.declare_dram_parameter("w4mlu", [D, 1], F32, isOutput=False)
    out_ext = nc.declare_dram_parameter("out", [BPC, 4 * D, LC], F32R, isOutput=True)

    with ExitStack() as ctx:
        tc = ctx.enter_context(tile.TileContext(nc))
        singles = ctx.enter_context(tc.tile_pool(name="singles", bufs=1))
        io = ctx.enter_context(tc.tile_pool(name="io", bufs=IO_BUFS))
        work = ctx.enter_context(tc.tile_pool(name="work", bufs=WORK_BUFS))
        psum = ctx.enter_context(tc.tile_pool(name="psum", bufs=1, space="PSUM"))

        ident_f = singles.tile([128, 128], F32)
        nc.vector.memset(ident_f, 1.0)
        ones = singles.tile([128, 128], F32R)
        nc.vector.tensor_copy(out=ones, in_=ident_f)
        nc.gpsimd.memset(ident_f, 0.0)
        nc.gpsimd.affine_select(
            out=ident_f, in_=ident_f, compare_op=ALU.not_equal, fill=1.0,
            base=0, pattern=[[-1, 128]], channel_multiplier=1)
        ident = singles.tile([128, 128], F32R)
        nc.vector.tensor_copy(out=ident, in_=ident_f)
        w4mlu_sb = singles.tile([128, 1], F32)
        nc.scalar.dma_start(out=w4mlu_sb, in_=w4mlu_in[:])
        w4C_sb = singles.tile([128, 1], F32)
        nc.scalar.dma_start(out=w4C_sb, in_=w4C_in[:])
        w4Q_sb = singles.tile([128, 1], F32)
        nc.scalar.dma_start(out=w4Q_sb, in_=w4Q_in[:])

        def prologue(b):
            """Input DMAs + augmented operands for batch b (halved for
            faster first-use)."""
            Qsb = io.tile([128, LQ], F32R, tag="Qsb")
            nc.sync.dma_start(out=Qsb, in_=Q_in[b])
            Csb = io.tile([128, LC], F32R, tag="Csb")
            for h in range(4):
                nc.sync.dma_start(
                    out=Csb[:, 512 * h:512 * (h + 1)],
                    in_=C_in[b, :, 512 * h:512 * (h + 1)])
            # C passthrough: pure DRAM->DRAM, independent of compute
            nc.sync.dma_start(out=out_ext[b, 0:128, :], in_=C_in[b])
            Qaug = work.tile([128, LQ], F32R, tag="Qaug")
            nc.vector.tensor_scalar(
                out=Qaug, in0=Qsb, scalar1=w4mlu_sb, scalar2=w4C_sb,
                op0=ALU.mult, op1=ALU.add)
            return dict(Qsb=Qsb, Csb=Csb, Qaug=Qaug)

        def caug_stage(b, st):
            Csb = st["Csb"]
            Caug = work.tile([128, LC], F32R, tag="Caug")
            for h in range(2):
                nc.vector.tensor_scalar(
                    out=Caug[:, 1024 * h:1024 * (h + 1)],
                    in0=Csb[:, 1024 * h:1024 * (h + 1)],
                    scalar1=w4mlu_sb, scalar2=w4Q_sb,
                    op0=ALU.mult, op1=ALU.add)
            st.update(Caug=Caug)

        def head_a(b, st):
            """Qt + first E2 groups for batch b."""
            Qsb, Csb = st["Qsb"], st["Csb"]
            Qaug, Caug = st["Qaug"], st["Caug"]

            # ---- Qt = Q.T ----
            Qt = work.tile([128, LQ], F32R, tag="Qt")
            ps_qt = psum.tile([128, 512], F32R, tag="small", bufs=SMALL_BUFS)
            for j in range(2):
                nc.tensor.transpose(
                    ps_qt[:, 128 * j:128 * (j + 1)],
                    Qsb[:, 128 * j:128 * (j + 1)], ident)
            nc.scalar.copy(out=Qt, in_=ps_qt[:, 0:256])

            # ---- E2[c,q] = exp(C.T @ Qaug): c-tile j at cols 256j.
            #      First half emitted before E1 so Act exps E1 early
            #      enough for the r-sums to chase. ----
            E2 = work.tile([128, 16 * LQ], F32R, tag="E2")

            def e2_group(g):
                ps = psum.tile([128, 1024], F32, tag="big", bufs=BIG_BUFS)
                for j in range(4):
                    ctile = g * 4 + j
                    nc.tensor.matmul(
                        ps[:, 256 * j:256 * (j + 1)],
                        Csb[:, 128 * ctile:128 * (ctile + 1)], Qaug,
                        start=True, stop=True)
                nc.scalar.activation(
                    out=E2[:, 1024 * g:1024 * (g + 1)], in_=ps, func=AF.Exp)

            e2_group(0)
            e2_group(1)
            e2_group(2)
            st.update(Qt=Qt, E2=E2, e2_group=e2_group)

        def head_b(b, st):
            """E1 + last E2 group for batch b."""
            Qsb, Caug = st["Qsb"], st["Caug"]
            e2_group = st["e2_group"]

            # ---- E1^T[q,c] = exp(Q.T @ Caug), g outer so r can chase ----
            E1 = work.tile([128, 2 * LC], F32R, tag="E1")
            for g in range(2):
                for qt in range(2):
                    ps = psum.tile([128, 1024], F32, tag="big", bufs=BIG_BUFS)
                    for cc in range(2):
                        c0 = 1024 * g + 512 * cc
                        nc.tensor.matmul(
                            ps[:, 512 * cc:512 * (cc + 1)],
                            Qsb[:, 128 * qt:128 * (qt + 1)],
                            Caug[:, c0:c0 + 512],
                            start=True, stop=True)
                    nc.scalar.activation(
                        out=E1[:, 2048 * qt + 1024 * g:2048 * qt + 1024 * (g + 1)],
                        in_=ps, func=AF.Exp)

            e2_group(3)
            st.update(E1=E1)

        def head_r(b, st):
            """Row-normalizer r -> rbi, Crbi; output tile alloc."""
            Csb, E1 = st["Csb"], st["E1"]

            # ---- r (replicated row-sums of E1 over q) -> rbi = 1/r;
            #      Crbi = C*(1/r) per half on gpsimd ----
            rbi = work.tile([128, LC], F32, tag="rbi")
            Crbi = work.tile([128, LC], F32, tag="Crbi")
            for g in range(2):
                sl = slice(1024 * g, 1024 * (g + 1))
                ps = psum.tile([128, 1024], F32, tag="big", bufs=BIG_BUFS)
                for cc in range(2):
                    c0 = 1024 * g + 512 * cc
                    for qt in range(2):
                        nc.tensor.matmul(
                            ps[:, 512 * cc:512 * (cc + 1)],
                            ones, E1[:, 2048 * qt + c0:2048 * qt + c0 + 512],
                            start=(qt == 0), stop=(qt == 1))
                nc.vector.reciprocal_approx_fast(out=rbi[:, sl], in_=ps)
                nc.gpsimd.tensor_mul(
                    out=Crbi[:, sl], in0=Csb[:, sl], in1=rbi[:, sl])

            outs = io.tile([128, 3, LC], F32R, tag="outs", name="outs")
            st.update(rbi=rbi, Crbi=Crbi, outs=outs)

        def core_b(b, st):
            """MA = Qt.T @ E1^T; out1/out2 muls + stores per chunk."""
            Qt, E1, Csb = st["Qt"], st["E1"], st["Csb"]
            rbi, Crbi, outs = st["rbi"], st["Crbi"], st["outs"]
            w = LC // OUT_CHUNKS
            for g in range(2):
                ps = psum.tile([128, 1024], F32, tag="big", bufs=BIG_BUFS)
                for cc in range(2):
                    c0 = 1024 * g + 512 * cc
                    for qt in range(2):
                        nc.tensor.matmul(
                            ps[:, 512 * cc:512 * (cc + 1)],
                            Qt[:, 128 * qt:128 * (qt + 1)],
                            E1[:, 2048 * qt + c0:2048 * qt + c0 + 512],
                            start=(qt == 0), stop=(qt == 1))
                for k in range(g * OUT_CHUNKS // 2, (g + 1) * OUT_CHUNKS // 2):
                    sl = slice(k * w, (k + 1) * w)
                    psl = slice(k * w - 1024 * g, (k + 1) * w - 1024 * g)
                    nc.vector.tensor_mul(
                        out=outs[:, 0, sl], in0=ps[:, psl], in1=rbi[:, sl])
                    nc.vector.tensor_mul(
                        out=outs[:, 1, sl], in0=ps[:, psl], in1=Crbi[:, sl])
                    nc.sync.dma_start(
                        out=out_ext[b, 128:256, sl], in_=outs[:, 0, sl])
                    nc.sync.dma_start(
                        out=out_ext[b, 256:384, sl], in_=outs[:, 1, sl])

        def c_ct(b, st):
            """Ct = C.T for batch b."""
            Csb = st["Csb"]

            # ---- Ct = C.T (16 transposes, col block j holds c-tile j) ----
            Ct = work.tile([128, LC], F32R, tag="Ct", bufs=1)
            for g in range(2):
                ps_ct = psum.tile([128, 1024], F32R, tag="big", bufs=BIG_BUFS)
                for j in range(8):
                    cj = g * 8 + j
                    nc.tensor.transpose(
                        ps_ct[:, 128 * j:128 * (j + 1)],
                        Csb[:, 128 * cj:128 * (cj + 1)], ident)
                nc.scalar.copy(out=Ct[:, 1024 * g:1024 * (g + 1)], in_=ps_ct)
            st.update(Ct=Ct)

        def c_t(b, st):
            """s sums + MT^T + T for batch b."""
            E2, Ct = st["E2"], st["Ct"]

            # ---- s (col-sums of E2 over c, replicated) -> sinv[q] compact ----
            s_sb = work.tile([128, LQ], F32, tag="s_sb")
            ps_s = psum.tile([128, 512], F32, tag="small", bufs=SMALL_BUFS)
            for j in range(16):
                nc.tensor.matmul(
                    ps_s[:, 0:256], ones, E2[:, 256 * j:256 * (j + 1)],
                    start=(j == 0), stop=(j == 15))
            nc.scalar.copy(out=s_sb, in_=ps_s[:, 0:256])
            ps_st = ps_s[:, 256:512]
            for j in range(2):
                nc.tensor.transpose(
                    ps_st[:, 128 * j:128 * (j + 1)],
                    s_sb[:, 128 * j:128 * (j + 1)], ident_f)

            # ---- MT^T = Ct.T @ E2 accumulated over c-tiles -> T = MT*sinv ----
            MTt = work.tile([128, LQ], F32R, tag="MTt")
            ps_mt = psum.tile([128, 512], F32, tag="small", bufs=SMALL_BUFS)
            for j in range(16):
                nc.tensor.matmul(
                    ps_mt[:, 0:256],
                    Ct[:, 128 * j:128 * (j + 1)], E2[:, 256 * j:256 * (j + 1)],
                    start=(j == 0), stop=(j == 15))
            nc.vector.tensor_copy(out=MTt, in_=ps_mt[:, 0:256])
            sinv = work.tile([128, 2], F32, tag="sinv")
            for j in range(2):
                nc.vector.reciprocal(
                    out=sinv[:, j:j + 1], in_=ps_st[:, 128 * j:128 * j + 1])
            st.update(MTt=MTt, sinv=sinv)

        def c_tt(b, st):
            """T = MTt.T * (1/s): emitted after head_b' so the waiting
            transpose/scale chain doesn't clog PE's reorder window."""
            MTt, sinv = st["MTt"], st["sinv"]
            T_sb = work.tile([128, LQ], F32R, tag="T_sb")
            ps_t = psum.tile([128, 512], F32R, tag="small", bufs=SMALL_BUFS)
            for j in range(2):
                nc.tensor.transpose(
                    ps_t[:, 128 * j:128 * (j + 1)],
                    MTt[:, 128 * j:128 * (j + 1)], ident)
            for j in range(2):
                nc.vector.tensor_scalar(
                    out=T_sb[:, 128 * j:128 * (j + 1)],
                    in0=ps_t[:, 128 * j:128 * (j + 1)],
                    scalar1=sinv[:, j:j + 1], scalar2=None,
                    op0=ALU.mult)
            st.update(T_sb=T_sb)

        def c_mb(b, st):
            """MB + out3 muls + stores for batch b."""
            E1, T_sb = st["E1"], st["T_sb"]
            Crbi, outs = st["Crbi"], st["outs"]

            # ---- MB = T.T @ E1^T in 512-wide chunks on the small psum
            #      pool: each chunk releases to its out3 mul + store without
            #      coupling the big-psum rotation to the slow end-of-batch
            #      readers. ----
            for k in range(4):
                c0 = 512 * k
                ps2 = psum.tile([128, 512], F32, tag="small", bufs=SMALL_BUFS)
                for qt in range(2):
                    nc.tensor.matmul(
                        ps2[:, 0:512],
                        T_sb[:, 128 * qt:128 * (qt + 1)],
                        E1[:, 2048 * qt + c0:2048 * qt + c0 + 512],
                        start=(qt == 0), stop=(qt == 1))
                sl = slice(c0, c0 + 512)
                nc.vector.tensor_mul(
                    out=outs[:, 2, sl], in0=ps2[:, 0:512], in1=Crbi[:, sl])
                nc.sync.dma_start(
                    out=out_ext[b, 384:512, sl], in_=outs[:, 2, sl])

        loop_cm = (tc.For_i(0, reps, 1,
                            hint_engines=(mybir.EngineType.PE,
                                          mybir.EngineType.DVE,
                                          mybir.EngineType.Activation,
                                          mybir.EngineType.SP,
                                          mybir.EngineType.Pool))
                   if hw_loop else nullcontext(0))
        with loop_cm:
         for rep in range(1 if hw_loop else reps):
          states = {}
          states[0] = st0 = prologue(0)
          caug_stage(0, st0)
          head_a(0, st0)
          if BPC > 1:
              states[1] = prologue(1)
              caug_stage(1, states[1])
          head_b(0, st0)
          if BPC > 1:
              head_a(1, states[1])
          head_r(0, st0)
          for b in range(BPC):
              st = states.pop(b)
              stn = states.get(b + 1)
              if b + 1 < BPC and stn is None:
                  states[b + 1] = stn = prologue(b + 1)
                  core_b(b, st)
                  caug_stage(b + 1, stn)
              else:
                  core_b(b, st)
              c_ct(b, st)
              c_t(b, st)
              if stn is not None:
                  if b >= 1:
                      head_a(b + 1, stn)
                  head_b(b + 1, stn)
              c_tt(b, st)
              c_mb(b, st)
              if stn is not None:
                  head_r(b + 1, stn)

    nc.compile()
    return nc


_NC = {}


def _get_nc(reps=1, hw_loop=False):
    key = (reps, hw_loop)
    if key not in _NC:
        _NC[key] = build_nc(reps, hw_loop)
    return _NC[key]


def make_in_maps(C, Q, w4C, w4Q, w4mlu):
    C = np.ascontiguousarray(np.asarray(C), dtype=np.float32)
    Q = np.ascontiguousarray(np.asarray(Q), dtype=np.float32)
    w4C = np.ascontiguousarray(np.asarray(w4C), dtype=np.float32).reshape(D, 1)
    w4Q = np.ascontiguousarray(np.asarray(w4Q), dtype=np.float32).reshape(D, 1)
    w4mlu = np.ascontiguousarray(np.asarray(w4mlu), dtype=np.float32).reshape(D, 1)
    in_maps = []
    for i in range(NCORES):
        sl = slice(i * BPC, (i + 1) * BPC)
        in_maps.append({
            "C": np.ascontiguousarray(C[sl]),
            "Q": np.ascontiguousarray(Q[sl]),
            "w4C": w4C, "w4Q": w4Q, "w4mlu": w4mlu,
        })
    return in_maps


def run(C, Q, w4C, w4Q, w4mlu, trace=False, tmpdir=None):
    from concourse.bass_utils import run_bass_kernel_spmd
    nc = _get_nc()
    in_maps = make_in_maps(C, Q, w4C, w4Q, w4mlu)
    res = run_bass_kernel_spmd(
        nc, in_maps, list(range(NCORES)), trace=trace, tmpdir=tmpdir)
    out = np.concatenate(
        [res.results[i]["out"] for i in range(NCORES)], axis=0)
    return out, res


def kernel(C, Q, Cmask=None, Qmask=None, w4C=None, w4Q=None, w4mlu=None,
           bias=None, **_unused):
    # Cmask/Qmask are all-ones in this problem and bias cancels exactly in
    # every output (softmax shift invariance), so neither reaches the device.
    out, _ = run(C, Q, w4C, w4Q, w4mlu)
    return out
